# revision 1
# baseline (speedup 1.0000x reference)
"""HAN layer (4-metapath GAT + semantic attention) for Trainium2, 8 NeuronCores.

Sharding: core c handles metapath m = c % 4, node-half h = c // 4
(N=30000 nodes -> two halves of 15000, padded to 15104 = 118 * 128).
Each core computes its feature projection feat = hs[m][half] @ W[m] on the
tensor engine ([15104,128] @ [128,256] as 118 PSUM-tile matmuls).
The data-dependent edge phase (edge softmax + neighborhood aggregation) and
the tiny semantic-attention reduction run on the host over the
device-computed projections.
"""
import sys
import numpy as np

sys.path.insert(0, "/opt/trn_rl_repo")

N, E, IN, H, D = 30000, 300000, 128, 4, 64
HD = H * D                      # 256
M = 4                           # metapaths
NCORES = 8
HALF = N // 2                   # 15000
HPAD = 15104                    # 118 * 128
P = 128
NT = HPAD // P                  # 118 tiles per core
NEG_ATTN = 0.2
NEG_ACT = 0.01


def _build_bass():
    import concourse.bacc as bacc
    import concourse.tile as tile
    from concourse import mybir
    from contextlib import ExitStack

    nc = bacc.Bacc()
    hsT = nc.declare_dram_parameter("hsT", (P, HPAD), mybir.dt.float32, isOutput=False)
    Wm = nc.declare_dram_parameter("Wm", (P, HD), mybir.dt.float32, isOutput=False)
    feat = nc.declare_dram_parameter("feat", (HPAD, HD), mybir.dt.float32, isOutput=True)

    with tile.TileContext(nc) as tc, ExitStack() as ctx:
        sb = ctx.enter_context(tc.tile_pool(name="sb", bufs=3))
        ps = ctx.enter_context(tc.tile_pool(name="ps", bufs=2, space="PSUM"))
        w_sb = sb.tile([P, HD], mybir.dt.float32, tag="w")
        nc.sync.dma_start(out=w_sb[:], in_=Wm[:, :])
        for t in range(NT):
            lhsT = sb.tile([P, P], mybir.dt.float32, tag="lhsT")
            nc.sync.dma_start(out=lhsT[:], in_=hsT[:, t * P:(t + 1) * P])
            acc = ps.tile([P, HD], mybir.dt.float32, space="PSUM", tag="acc")
            nc.tensor.matmul(out=acc[:], lhsT=lhsT[:], rhs=w_sb[:],
                             start=True, stop=True)
            stg = sb.tile([P, HD], mybir.dt.float32, tag="stg")
            nc.scalar.copy(out=stg[:], in_=acc[:])
            nc.sync.dma_start(out=feat[t * P:(t + 1) * P, :], in_=stg[:])
    nc.compile()
    return nc


def _run_device(hs, W):
    """Returns feats[m] = hs[m] @ W[m] as [N, HD], computed on 8 neuron cores."""
    from concourse.bass_utils import run_bass_kernel_spmd

    nc = _build_bass()
    in_maps = []
    for c in range(NCORES):
        m, h = c % M, c // M
        hs_half = hs[m][h * HALF:(h + 1) * HALF]            # [15000, 128]
        hsT = np.zeros((P, HPAD), np.float32)
        hsT[:, :HALF] = hs_half.T
        in_maps.append({"hsT": np.ascontiguousarray(hsT),
                        "Wm": np.ascontiguousarray(W[m])})
    res = run_bass_kernel_spmd(nc, in_maps, list(range(NCORES)))
    feats = []
    for m in range(M):
        top = res.results[m]["feat"][:HALF]
        bot = res.results[m + 4]["feat"][:HALF]
        feats.append(np.concatenate([top, bot], axis=0))    # [N, HD]
    return feats, res


def _gat_edge_phase(featm, src, dst, al, ar, b):
    """Edge softmax + aggregation, numerically identical to the reference
    (alpha = ex/sum(ex) is invariant to the max-shift; |e| < ~3 so exp is safe)."""
    f = featm.reshape(N, H, D)
    el = (f * al).sum(-1)                                   # [N, H]
    er = (f * ar).sum(-1)
    e = el[src] + er[dst]
    e = np.where(e > 0, e, NEG_ATTN * e)                    # leaky_relu 0.2
    ex = np.exp(e)                                          # [E, H]
    order = np.argsort(dst, kind="stable")
    ds = dst[order]
    starts = np.flatnonzero(np.r_[True, ds[1:] != ds[:-1]])
    uniq = ds[starts]
    exs = ex[order]
    den = np.add.reduceat(exs, starts, axis=0)              # [U, H]
    msg = f[src[order]] * exs[:, :, None]                   # [E, H, D]
    sums = np.add.reduceat(msg.reshape(E, HD), starts, axis=0)
    out = np.zeros((N, H, D), np.float32)
    out[uniq] = sums.reshape(-1, H, D) / np.maximum(den, 1e-9)[:, :, None]
    out = out + b.reshape(1, H, D)
    out = np.where(out > 0, out, NEG_ACT * out)             # leaky_relu 0.01
    return out.reshape(N, HD).astype(np.float32)


def _semantic(z, Wp1, bp1, Wp2):
    w = (np.tanh(z @ Wp1 + bp1) @ Wp2).mean(0)              # [2, 1]
    w = w - w.max()
    beta = np.exp(w) / np.exp(w).sum()
    return (beta[None] * z).sum(1)


def kernel(hs, src, dst, W, attn_l, attn_r, bias, Wp1, bp1, Wp2):
    hs = np.asarray(hs, np.float32)
    src = np.asarray(src)
    dst = np.asarray(dst)
    W = np.asarray(W, np.float32)

    feats, _ = _run_device(hs, W)

    outs = []
    for m in range(M):
        outs.append(_gat_edge_phase(feats[m], src[m].astype(np.int64),
                                    dst[m].astype(np.int64),
                                    np.asarray(attn_l[m]), np.asarray(attn_r[m]),
                                    np.asarray(bias[m])))
    Wp1 = np.asarray(Wp1); bp1 = np.asarray(bp1); Wp2 = np.asarray(Wp2)
    lnc = _semantic(np.stack([outs[1], outs[2]], axis=1), Wp1, bp1, Wp2)
    dis = _semantic(np.stack([outs[0], outs[3]], axis=1), Wp1, bp1, Wp2)
    return np.stack([lnc, dis]).astype(np.float32)



# revision 6
# speedup vs baseline: 2688.4527x; 2688.4527x over previous
"""HAN layer (4-metapath GAT + semantic attention) on 8 Trainium2 NeuronCores.

Sharding: core c handles metapath m = c % 4 and node-half h = c // 4
(235 blocks of 128 nodes; h=0 -> blocks 0..118, h=1 -> blocks 118..235 + dummy).

Everything heavy runs on device in ONE NEFF per core:
 - feature projection feat = hs @ W (bf16, tensor engine) -> fx in DRAM
 - per node-block: one indirect dma_gather pulls feat[src] for all the
   block's (sorted, padded) edges as [128, T, 256] (edge j -> partition
   j%128, slice j//128)
 - one-hot scatter matmuls: acc[n, :] += sum_e onehot[e,n] * (feat[src_e]
   * P_e | P_e), accumulating both messages and softmax denominators in a
   single PSUM tile ([128, 260])
 - epilogue: out = leaky_relu(acc/den + bias) on the vector engine

Host does only O(E) index prep: edge attention logits el/er (tiny matmuls),
P = exp(leaky_relu(el[src]+er[dst])), sort by dst, pack per-block layouts.
Softmax max-shift is skipped: alpha = P/sum(P) is shift-invariant and |e|<~3.
Semantic attention (beta softmax over 2 metapaths + combine) runs on host.
"""
import sys
import numpy as np

sys.path.insert(0, "/opt/trn_rl_repo")

N, E, IN, H, D = 30000, 300000, 128, 4, 64
HD = H * D                      # 256
M = 4                           # metapaths
NCORES = 8
P = 128
NBLK = (N + P - 1) // P         # 235 node blocks total
NPAD = NBLK * P                 # 30080
NB = (NBLK + 1) // 2            # 118 blocks per core
T_MIN = 12                      # min tiles (of 128 edges) per block
NEG_ATTN = 0.2
NEG_ACT = 0.01

_NC_CACHE = {}


def _build_nc(T, nb=NB, nblk=NBLK, npad=NPAD):
    """One-core program; same NEFF runs SPMD on all 8 cores."""
    import concourse.bacc as bacc
    import concourse.tile as tile
    from concourse import mybir
    from contextlib import ExitStack

    f32 = mybir.dt.float32
    bf16 = mybir.dt.bfloat16
    i16 = mybir.dt.int16
    AOP = mybir.AluOpType

    nc = bacc.Bacc()
    hsT = nc.declare_dram_parameter("hsT", (P, npad), bf16, isOutput=False)
    Wm = nc.declare_dram_parameter("Wm", (P, HD), bf16, isOutput=False)
    idx = nc.declare_dram_parameter("idx", (P, nb * T * 8), i16, isOutput=False)
    Pb = nc.declare_dram_parameter("Pb", (P, nb * T * H), bf16, isOutput=False)
    dstl = nc.declare_dram_parameter("dstl", (P, nb * T), bf16, isOutput=False)
    brep = nc.declare_dram_parameter("brep", (P, HD), f32, isOutput=False)
    iotaf = nc.declare_dram_parameter("iotaf", (P, P), bf16, isOutput=False)
    outp = nc.declare_dram_parameter("outp", (nb * P, HD), f32, isOutput=True)

    with tile.TileContext(nc) as tc, ExitStack() as ctx:
        const = ctx.enter_context(tc.tile_pool(name="const", bufs=1))
        dram = ctx.enter_context(tc.tile_pool(name="dram", bufs=1, space="DRAM"))
        sb = ctx.enter_context(tc.tile_pool(name="sb", bufs=3))
        gp = ctx.enter_context(tc.tile_pool(name="gp", bufs=2))
        ps = ctx.enter_context(tc.tile_pool(name="ps", bufs=2, space="PSUM"))

        fx = dram.tile([npad, HD], bf16, tag="fx")

        w_sb = const.tile([P, HD], bf16, tag="w")
        nc.sync.dma_start(out=w_sb[:], in_=Wm[:, :])
        iota_sb = const.tile([P, P], bf16, tag="iota")
        nc.sync.dma_start(out=iota_sb[:], in_=iotaf[:, :])
        brep_sb = const.tile([P, HD], f32, tag="brep")
        nc.sync.dma_start(out=brep_sb[:], in_=brep[:, :])
        idx_sb = const.tile([P, nb * T * 8], i16, tag="idx")
        nc.sync.dma_start(out=idx_sb[:], in_=idx[:, :])
        pb_sb = const.tile([P, nb * T * H], bf16, tag="pb")
        nc.sync.dma_start(out=pb_sb[:], in_=Pb[:, :])
        dl_sb = const.tile([P, nb * T], bf16, tag="dl")
        nc.sync.dma_start(out=dl_sb[:], in_=dstl[:, :])
        hsT_sb = const.tile([P, npad], bf16, tag="hsT")
        # split the 60KB/partition load into chunks so it pipelines
        hchunk = -(-npad // 5)
        for k0 in range(0, npad, hchunk):
            k1 = min(k0 + hchunk, npad)
            nc.sync.dma_start(out=hsT_sb[:, k0:k1], in_=hsT[:, k0:k1])

        # ---- phase A: projection feat = hsT^T @ W -> fx (bf16) ----
        SGRP = 4                          # proj tiles per staging buffer
        for g in range(nblk // SGRP + (1 if nblk % SGRP else 0)):
            n_in_g = min(SGRP, nblk - g * SGRP)
            stg = sb.tile([P, SGRP, HD], bf16, tag="stg")
            for j in range(n_in_g):
                i = g * SGRP + j
                pacc = ps.tile([P, HD], f32, tag="pacc")
                nc.tensor.matmul(out=pacc[:], lhsT=hsT_sb[:, i * P:(i + 1) * P],
                                 rhs=w_sb[:], start=True, stop=True)
                nc.scalar.copy(out=stg[:, j, :], in_=pacc[:])
            rows = fx[g * SGRP * P:(g * SGRP + n_in_g) * P, :]
            nc.sync.dma_start(
                out=rows.rearrange("(j p) d -> p j d", p=P),
                in_=stg[:, 0:n_in_g, :])

        # ---- phase B: per-block gather + scatter-matmul + epilogue ----
        for i in range(nb):
            G = gp.tile([P, T, HD], bf16, tag="G")
            # SWDGE descriptor carveout holds 1024 descs; chunk the gather
            GCH = 6
            for t0 in range(0, T, GCH):
                t1 = min(t0 + GCH, T)
                nc.gpsimd.dma_gather(
                    out_ap=G[:, t0:t1, :], in_ap=fx[:, :],
                    idxs_ap=idx_sb[:, i * T * 8 + t0 * 8:i * T * 8 + t1 * 8],
                    num_idxs=(t1 - t0) * P, num_idxs_reg=(t1 - t0) * P,
                    elem_size=HD)

            oh = sb.tile([P, T, P], bf16, tag="oh")
            nc.vector.tensor_tensor(
                out=oh[:, :, :],
                in0=iota_sb[:, :].unsqueeze(1).broadcast_to([P, T, P]),
                in1=dl_sb[:, i * T:(i + 1) * T].unsqueeze(2).broadcast_to([P, T, P]),
                op=AOP.is_equal)

            rhs = sb.tile([P, T, HD + H], bf16, tag="rhs")
            nc.vector.tensor_tensor(
                out=rhs[:, :, 0:HD].rearrange("p t (h d) -> p t h d", h=H),
                in0=G[:, :, :].rearrange("p t (h d) -> p t h d", h=H),
                in1=pb_sb[:, i * T * H:(i + 1) * T * H]
                    .rearrange("p (t h) -> p t h", t=T)
                    .unsqueeze(3).broadcast_to([P, T, H, D]),
                op=AOP.mult)
            nc.scalar.copy(
                out=rhs[:, :, HD:HD + H],
                in_=pb_sb[:, i * T * H:(i + 1) * T * H]
                    .rearrange("p (t h) -> p t h", t=T))

            acc = ps.tile([P, HD + H], f32, tag="acc")
            for t in range(T):
                nc.tensor.matmul(out=acc[:], lhsT=oh[:, t, :], rhs=rhs[:, t, :],
                                 start=(t == 0), stop=(t == T - 1))

            den = sb.tile([P, H], f32, tag="den")
            nc.vector.tensor_scalar(out=den[:], in0=acc[:, HD:HD + H],
                                    scalar1=1e-9, scalar2=None, op0=AOP.max)
            rden = sb.tile([P, H], f32, tag="rden")
            nc.vector.reciprocal(out=rden[:], in_=den[:])
            outb = sb.tile([P, HD], f32, tag="outb")
            for h in range(H):
                nc.vector.scalar_tensor_tensor(
                    out=outb[:, h * D:(h + 1) * D],
                    in0=acc[:, h * D:(h + 1) * D],
                    scalar=rden[:, h:h + 1],
                    in1=brep_sb[:, h * D:(h + 1) * D],
                    op0=AOP.mult, op1=AOP.add)
            outb2 = sb.tile([P, HD], f32, tag="outb2")
            nc.vector.scalar_tensor_tensor(
                out=outb2[:], in0=outb[:], scalar=NEG_ACT, in1=outb[:],
                op0=AOP.mult, op1=AOP.max)
            nc.sync.dma_start(out=outp[i * P:(i + 1) * P, :], in_=outb2[:])

    nc.compile()
    return nc


def _get_nc(T):
    if T not in _NC_CACHE:
        _NC_CACHE[T] = _build_nc(T)
    return _NC_CACHE[T]


def _attn_mat(a):
    """[H, D] head vectors -> [HD, H] block-diagonal matrix."""
    A = np.zeros((HD, H), np.float32)
    for h in range(H):
        A[h * D:(h + 1) * D, h] = a[h]
    return A


def _prep_metapath(hs_m, src_m, dst_m, W_m, al_m, ar_m):
    """Edge exp-weights + dst-sorted edge arrays for one metapath."""
    Wel = (W_m @ _attn_mat(al_m)).astype(np.float32)     # [IN, H]
    Wer = (W_m @ _attn_mat(ar_m)).astype(np.float32)
    el = hs_m @ Wel                                       # [N, H]
    er = hs_m @ Wer
    e = el[src_m] + er[dst_m]                             # [E, H]
    e = np.where(e > 0, e, NEG_ATTN * e)
    Pw = np.exp(e).astype(np.float32)
    order = np.argsort(dst_m, kind="stable")
    ss = src_m[order].astype(np.int64)
    ds = dst_m[order].astype(np.int64)
    Ps = Pw[order]
    blk = ds >> 7
    counts = np.bincount(blk, minlength=NBLK)
    starts = np.concatenate([[0], np.cumsum(counts)[:-1]])
    return ss, ds, Ps, counts, starts


def _pack_core(ss, ds, Ps, counts, starts, blocks, T):
    """Device-layout inputs for one core's list of node blocks."""
    import ml_dtypes
    bf16 = ml_dtypes.bfloat16
    SLOTS = T * P
    nb = NB
    src_pad = np.zeros((nb, SLOTS), np.int64)
    P_pad = np.zeros((nb, SLOTS, H), np.float32)
    dl_pad = np.zeros((nb, SLOTS), np.float32)
    for i, b in enumerate(blocks):
        if b is None:
            continue
        c = counts[b]
        s0 = starts[b]
        src_pad[i, :c] = ss[s0:s0 + c]
        P_pad[i, :c] = Ps[s0:s0 + c]
        dl_pad[i, :c] = ds[s0:s0 + c] - (b << 7)
    # gather index tile: index j -> (partition j%16, col j//16), replicated
    w16 = src_pad.reshape(nb, T * 8, 16)
    idx16 = np.tile(
        w16.transpose(2, 0, 1).reshape(16, nb * T * 8), (8, 1)).astype(np.int16)
    # edge j -> (partition j%128, tile j//128)
    Pt = P_pad.reshape(nb, T, P, H).transpose(2, 0, 1, 3).reshape(P, nb * T * H)
    dlt = dl_pad.reshape(nb, T, P).transpose(2, 0, 1).reshape(P, nb * T)
    return (np.ascontiguousarray(idx16),
            np.ascontiguousarray(Pt.astype(bf16)),
            np.ascontiguousarray(dlt.astype(bf16)))


def _run_device(hs, src, dst, W, attn_l, attn_r, bias, trace=False):
    import ml_dtypes
    from concourse.bass_utils import run_bass_kernel_spmd
    bf16 = ml_dtypes.bfloat16

    preps = [_prep_metapath(np.asarray(hs[m], np.float32), src[m], dst[m],
                            np.asarray(W[m], np.float32),
                            np.asarray(attn_l[m]), np.asarray(attn_r[m]))
             for m in range(M)]
    maxc = max(int(pr[3].max()) for pr in preps)
    T = max(T_MIN, -(-maxc // P))
    nc = _get_nc(T)

    iota = np.ascontiguousarray(
        np.tile(np.arange(P, dtype=np.float32), (P, 1)).astype(bf16))
    in_maps = []
    for c in range(NCORES):
        m, h = c % M, c // M
        ss, ds, Ps, counts, starts = preps[m]
        blocks = (list(range(0, NB)) if h == 0
                  else list(range(NB, NBLK)) + [None])
        idx16, Pt, dlt = _pack_core(ss, ds, Ps, counts, starts, blocks, T)
        hsT = np.zeros((P, NPAD), np.float32)
        hsT[:, :N] = np.asarray(hs[m], np.float32).T
        in_maps.append({
            "hsT": np.ascontiguousarray(hsT.astype(bf16)),
            "Wm": np.ascontiguousarray(np.asarray(W[m]).astype(bf16)),
            "idx": idx16, "Pb": Pt, "dstl": dlt,
            "brep": np.ascontiguousarray(
                np.tile(np.asarray(bias[m], np.float32), (P, 1))),
            "iotaf": iota,
        })
    kw = {}
    if trace:
        kw = dict(trace=True, trace_cores=list(range(NCORES)))
    res = run_bass_kernel_spmd(nc, in_maps, list(range(NCORES)), **kw)
    outs = []
    for m in range(M):
        top = res.results[m]["outp"][:NB * P]
        bot = res.results[m + 4]["outp"][:(NBLK - NB) * P]
        outs.append(np.concatenate([top, bot], axis=0)[:N])
    return outs, res


def _semantic(z, Wp1, bp1, Wp2):
    w = (np.tanh(z @ Wp1 + bp1) @ Wp2).mean(0)            # [2, 1]
    w = w - w.max()
    beta = np.exp(w) / np.exp(w).sum()
    return (beta[None] * z).sum(1)


def kernel(hs, src, dst, W, attn_l, attn_r, bias, Wp1, bp1, Wp2):
    hs = np.asarray(hs, np.float32)
    src = np.asarray(src)
    dst = np.asarray(dst)
    W = np.asarray(W, np.float32)

    outs, _ = _run_device(hs, src, dst, W, attn_l, attn_r, bias)

    Wp1 = np.asarray(Wp1, np.float32)
    bp1 = np.asarray(bp1, np.float32)
    Wp2 = np.asarray(Wp2, np.float32)
    lnc = _semantic(np.stack([outs[1], outs[2]], axis=1), Wp1, bp1, Wp2)
    dis = _semantic(np.stack([outs[0], outs[3]], axis=1), Wp1, bp1, Wp2)
    return np.stack([lnc, dis]).astype(np.float32)


# revision 14
# speedup vs baseline: 3027.0719x; 1.1260x over previous
"""HAN layer (4-metapath GAT + semantic attention) on 8 Trainium2 NeuronCores.

Sharding: core c handles metapath m = c % 4 and node-half h = c // 4
(235 blocks of 128 nodes; h=0 -> blocks 0..118, h=1 -> blocks 118..235 + dummy).

Everything heavy runs on device in ONE NEFF per core:
 - feature projection feat = hs @ W (bf16, tensor engine) -> fx in DRAM
 - per node-block: one indirect dma_gather pulls feat[src] for all the
   block's (sorted, padded) edges as [128, T, 256] (edge j -> partition
   j%128, slice j//128)
 - one-hot scatter matmuls: acc[n, :] += sum_e onehot[e,n] * (feat[src_e]
   * P_e | P_e), accumulating both messages and softmax denominators in a
   single PSUM tile ([128, 260])
 - epilogue: out = leaky_relu(acc/den + bias) on the vector engine

Host does only O(E) index prep: edge attention logits el/er (tiny matmuls),
P = exp(leaky_relu(el[src]+er[dst])), sort by dst, pack per-block layouts.
Softmax max-shift is skipped: alpha = P/sum(P) is shift-invariant and |e|<~3.
Semantic attention (beta softmax over 2 metapaths + combine) runs on host.
"""
import sys
import numpy as np

sys.path.insert(0, "/opt/trn_rl_repo")

N, E, IN, H, D = 30000, 300000, 128, 4, 64
HD = H * D                      # 256
M = 4                           # metapaths
NCORES = 8
P = 128
NBLK = (N + P - 1) // P         # 235 node blocks total
NPAD = NBLK * P                 # 30080
NB = (NBLK + 1) // 2            # 118 blocks per core
T_MIN = 12                      # min tiles (of 128 edges) per block
NEG_ATTN = 0.2
NEG_ACT = 0.01

_NC_CACHE = {}


def _build_nc(Tb, nb=NB, nblk=NBLK, npad=NPAD):
    """One-core program; same NEFF runs SPMD on all 8 cores.

    Tb: per-block-index tile counts (128 edges per tile), the max over the
    8 cores — pad descriptors cost as much as real ones, so block slots are
    sized to what the cores actually need rather than a uniform worst case.
    """
    import concourse.bacc as bacc
    import concourse.tile as tile
    from concourse import mybir
    from contextlib import ExitStack

    f32 = mybir.dt.float32
    bf16 = mybir.dt.bfloat16
    i16 = mybir.dt.int16
    AOP = mybir.AluOpType

    assert len(Tb) == nb
    NT = sum(Tb)                      # total tiles across blocks
    off = [0]
    for t in Tb:
        off.append(off[-1] + t)

    nc = bacc.Bacc()
    hsT = nc.declare_dram_parameter("hsT", (P, npad), bf16, isOutput=False)
    Wm = nc.declare_dram_parameter("Wm", (P, HD), bf16, isOutput=False)
    idx = nc.declare_dram_parameter("idx", (P, NT * 8), i16, isOutput=False)
    Pb = nc.declare_dram_parameter("Pb", (P, NT * H), bf16, isOutput=False)
    dstl = nc.declare_dram_parameter("dstl", (P, NT), bf16, isOutput=False)
    brep = nc.declare_dram_parameter("brep", (P, HD), f32, isOutput=False)
    iotaf = nc.declare_dram_parameter("iotaf", (P, P), bf16, isOutput=False)
    outp = nc.declare_dram_parameter("outp", (nb * P, HD), f32, isOutput=True)

    with tile.TileContext(nc) as tc, ExitStack() as ctx:
        const = ctx.enter_context(tc.tile_pool(name="const", bufs=1))
        dram = ctx.enter_context(tc.tile_pool(name="dram", bufs=1, space="DRAM"))
        sb = ctx.enter_context(tc.tile_pool(name="sb", bufs=3))
        gp = ctx.enter_context(tc.tile_pool(name="gp", bufs=2))
        ps = ctx.enter_context(tc.tile_pool(name="ps", bufs=2, space="PSUM"))

        fx = dram.tile([npad, HD], bf16, tag="fx")

        w_sb = const.tile([P, HD], bf16, tag="w")
        nc.sync.dma_start(out=w_sb[:], in_=Wm[:, :])
        iota_sb = const.tile([P, P], bf16, tag="iota")
        nc.sync.dma_start(out=iota_sb[:], in_=iotaf[:, :])
        brep_sb = const.tile([P, HD], f32, tag="brep")
        nc.sync.dma_start(out=brep_sb[:], in_=brep[:, :])
        idx_sb = const.tile([P, NT * 8], i16, tag="idx")
        nc.sync.dma_start(out=idx_sb[:], in_=idx[:, :])
        pb_sb = const.tile([P, NT * H], bf16, tag="pb")
        nc.sync.dma_start(out=pb_sb[:], in_=Pb[:, :])
        dl_sb = const.tile([P, NT], bf16, tag="dl")
        nc.sync.dma_start(out=dl_sb[:], in_=dstl[:, :])
        hsT_sb = const.tile([P, npad], bf16, tag="hsT")
        # split the 60KB/partition load into chunks so it pipelines
        hchunk = -(-npad // 5)
        for k0 in range(0, npad, hchunk):
            k1 = min(k0 + hchunk, npad)
            nc.sync.dma_start(out=hsT_sb[:, k0:k1], in_=hsT[:, k0:k1])

        # ---- phase A: projection feat = hsT^T @ W -> fx (bf16) ----
        SGRP = 4                          # proj tiles per staging buffer
        for g in range(nblk // SGRP + (1 if nblk % SGRP else 0)):
            n_in_g = min(SGRP, nblk - g * SGRP)
            stg = sb.tile([P, SGRP, HD], bf16, tag="stg")
            for j in range(n_in_g):
                i = g * SGRP + j
                pacc = ps.tile([P, HD], f32, tag="pacc")
                nc.tensor.matmul(out=pacc[:], lhsT=hsT_sb[:, i * P:(i + 1) * P],
                                 rhs=w_sb[:], start=True, stop=True)
                if i % 2 == 0:
                    nc.scalar.copy(out=stg[:, j, :], in_=pacc[:])
                else:
                    nc.vector.tensor_copy(out=stg[:, j, :], in_=pacc[:])
            rows = fx[g * SGRP * P:(g * SGRP + n_in_g) * P, :]
            nc.sync.dma_start(
                out=rows.rearrange("(j p) d -> p j d", p=P),
                in_=stg[:, 0:n_in_g, :])

        # ---- phase B: per-block gather + scatter-matmul + epilogue ----
        TMAX = max(Tb)
        for i in range(nb):
            T = Tb[i]
            o = off[i]
            G = gp.tile([P, TMAX, HD], bf16, tag="G")
            # SWDGE descriptor carveout holds 1024 descs; chunk the gather
            GCH = 6
            for t0 in range(0, T, GCH):
                t1 = min(t0 + GCH, T)
                nc.gpsimd.dma_gather(
                    out_ap=G[:, t0:t1, :], in_ap=fx[:, :],
                    idxs_ap=idx_sb[:, (o + t0) * 8:(o + t1) * 8],
                    num_idxs=(t1 - t0) * P, num_idxs_reg=(t1 - t0) * P,
                    elem_size=HD)

            oh = sb.tile([P, TMAX, P], bf16, tag="oh")
            nc.vector.tensor_tensor(
                out=oh[:, 0:T, :],
                in0=iota_sb[:, :].unsqueeze(1).broadcast_to([P, T, P]),
                in1=dl_sb[:, o:o + T].unsqueeze(2).broadcast_to([P, T, P]),
                op=AOP.is_equal)

            rhs = sb.tile([P, TMAX, HD + H], bf16, tag="rhs")
            nc.vector.tensor_tensor(
                out=rhs[:, 0:T, 0:HD].rearrange("p t (h d) -> p t h d", h=H),
                in0=G[:, 0:T, :].rearrange("p t (h d) -> p t h d", h=H),
                in1=pb_sb[:, o * H:(o + T) * H]
                    .rearrange("p (t h) -> p t h", t=T)
                    .unsqueeze(3).broadcast_to([P, T, H, D]),
                op=AOP.mult)
            nc.scalar.copy(
                out=rhs[:, 0:T, HD:HD + H],
                in_=pb_sb[:, o * H:(o + T) * H]
                    .rearrange("p (t h) -> p t h", t=T))

            acc = ps.tile([P, HD + H], f32, tag="acc")
            for t in range(T):
                nc.tensor.matmul(out=acc[:], lhsT=oh[:, t, :], rhs=rhs[:, t, :],
                                 start=(t == 0), stop=(t == T - 1))

            den = sb.tile([P, H], f32, tag="den")
            nc.vector.tensor_scalar(out=den[:], in0=acc[:, HD:HD + H],
                                    scalar1=1e-9, scalar2=None, op0=AOP.max)
            rden = sb.tile([P, H], f32, tag="rden")
            nc.vector.reciprocal(out=rden[:], in_=den[:])
            outb = sb.tile([P, HD], f32, tag="outb")
            for h in range(H):
                nc.vector.scalar_tensor_tensor(
                    out=outb[:, h * D:(h + 1) * D],
                    in0=acc[:, h * D:(h + 1) * D],
                    scalar=rden[:, h:h + 1],
                    in1=brep_sb[:, h * D:(h + 1) * D],
                    op0=AOP.mult, op1=AOP.add)
            outb2 = sb.tile([P, HD], f32, tag="outb2")
            nc.vector.scalar_tensor_tensor(
                out=outb2[:], in0=outb[:], scalar=NEG_ACT, in1=outb[:],
                op0=AOP.mult, op1=AOP.max)
            nc.sync.dma_start(out=outp[i * P:(i + 1) * P, :], in_=outb2[:])

    nc.compile()
    return nc


def _get_nc(Tb):
    if Tb not in _NC_CACHE:
        _NC_CACHE[Tb] = _build_nc(Tb)
    return _NC_CACHE[Tb]


def _attn_mat(a):
    """[H, D] head vectors -> [HD, H] block-diagonal matrix."""
    A = np.zeros((HD, H), np.float32)
    for h in range(H):
        A[h * D:(h + 1) * D, h] = a[h]
    return A


def _prep_metapath(hs_m, src_m, dst_m, W_m, al_m, ar_m):
    """Edge exp-weights + dst-sorted edge arrays for one metapath."""
    Wel = (W_m @ _attn_mat(al_m)).astype(np.float32)     # [IN, H]
    Wer = (W_m @ _attn_mat(ar_m)).astype(np.float32)
    el = hs_m @ Wel                                       # [N, H]
    er = hs_m @ Wer
    e = el[src_m] + er[dst_m]                             # [E, H]
    e = np.where(e > 0, e, NEG_ATTN * e)
    Pw = np.exp(e).astype(np.float32)
    order = np.argsort(dst_m, kind="stable")
    ss = src_m[order].astype(np.int64)
    ds = dst_m[order].astype(np.int64)
    Ps = Pw[order]
    blk = ds >> 7
    counts = np.bincount(blk, minlength=NBLK)
    starts = np.concatenate([[0], np.cumsum(counts)[:-1]])
    return ss, ds, Ps, counts, starts


def _pack_core(ss, ds, Ps, counts, starts, blocks, Tb):
    """Device-layout inputs for one core's list of node blocks (variable Tb)."""
    import ml_dtypes
    bf16 = ml_dtypes.bfloat16
    NT = sum(Tb)
    idx16 = np.zeros((16, NT * 8), np.int16)
    Pt = np.zeros((P, NT * H), np.float32)
    dlt = np.zeros((P, NT), np.float32)
    o = 0
    for i, b in enumerate(blocks):
        T = Tb[i]
        SLOTS = T * P
        src_pad = np.zeros(SLOTS, np.int64)
        P_pad = np.zeros((SLOTS, H), np.float32)
        dl_pad = np.zeros(SLOTS, np.float32)
        if b is not None:
            c = counts[b]
            s0 = starts[b]
            src_pad[:c] = ss[s0:s0 + c]
            P_pad[:c] = Ps[s0:s0 + c]
            dl_pad[:c] = ds[s0:s0 + c] - (b << 7)
        # gather idx: index j -> (partition j%16, col j//16)
        idx16[:, o * 8:(o + T) * 8] = src_pad.reshape(T * 8, 16).T
        # edge j -> (partition j%128, tile j//128)
        Pt[:, o * H:(o + T) * H] = (
            P_pad.reshape(T, P, H).transpose(1, 0, 2).reshape(P, T * H))
        dlt[:, o:o + T] = dl_pad.reshape(T, P).T
        o += T
    idx16 = np.tile(idx16, (8, 1)).astype(np.int16)
    return (np.ascontiguousarray(idx16),
            np.ascontiguousarray(Pt.astype(bf16)),
            np.ascontiguousarray(dlt.astype(bf16)))


def _run_device(hs, src, dst, W, attn_l, attn_r, bias, trace=False):
    import ml_dtypes
    from concourse.bass_utils import run_bass_kernel_spmd
    bf16 = ml_dtypes.bfloat16

    preps = [_prep_metapath(np.asarray(hs[m], np.float32), src[m], dst[m],
                            np.asarray(W[m], np.float32),
                            np.asarray(attn_l[m]), np.asarray(attn_r[m]))
             for m in range(M)]
    core_blocks = []
    for c in range(NCORES):
        h = c // M
        core_blocks.append(list(range(0, NB)) if h == 0
                           else list(range(NB, NBLK)) + [None])
    # per-block-index tile count = max need across the 8 cores
    Tb = []
    for i in range(NB):
        mx = 1
        for c in range(NCORES):
            b = core_blocks[c][i]
            if b is not None:
                mx = max(mx, -(-int(preps[c % M][3][b]) // P))
        Tb.append(mx)
    Tb = tuple(Tb)
    nc = _get_nc(Tb)

    iota = np.ascontiguousarray(
        np.tile(np.arange(P, dtype=np.float32), (P, 1)).astype(bf16))
    in_maps = []
    for c in range(NCORES):
        m, h = c % M, c // M
        ss, ds, Ps, counts, starts = preps[m]
        idx16, Pt, dlt = _pack_core(ss, ds, Ps, counts, starts,
                                    core_blocks[c], Tb)
        hsT = np.zeros((P, NPAD), np.float32)
        hsT[:, :N] = np.asarray(hs[m], np.float32).T
        in_maps.append({
            "hsT": np.ascontiguousarray(hsT.astype(bf16)),
            "Wm": np.ascontiguousarray(np.asarray(W[m]).astype(bf16)),
            "idx": idx16, "Pb": Pt, "dstl": dlt,
            "brep": np.ascontiguousarray(
                np.tile(np.asarray(bias[m], np.float32), (P, 1))),
            "iotaf": iota,
        })
    kw = {}
    if trace:
        kw = dict(trace=True, trace_cores=list(range(NCORES)))
    res = run_bass_kernel_spmd(nc, in_maps, list(range(NCORES)), **kw)
    outs = []
    for m in range(M):
        top = res.results[m]["outp"][:NB * P]
        bot = res.results[m + 4]["outp"][:(NBLK - NB) * P]
        outs.append(np.concatenate([top, bot], axis=0)[:N])
    return outs, res


def _semantic(z, Wp1, bp1, Wp2):
    w = (np.tanh(z @ Wp1 + bp1) @ Wp2).mean(0)            # [2, 1]
    w = w - w.max()
    beta = np.exp(w) / np.exp(w).sum()
    return (beta[None] * z).sum(1)


def kernel(hs, src, dst, W, attn_l, attn_r, bias, Wp1, bp1, Wp2):
    hs = np.asarray(hs, np.float32)
    src = np.asarray(src)
    dst = np.asarray(dst)
    W = np.asarray(W, np.float32)

    outs, _ = _run_device(hs, src, dst, W, attn_l, attn_r, bias)

    Wp1 = np.asarray(Wp1, np.float32)
    bp1 = np.asarray(bp1, np.float32)
    Wp2 = np.asarray(Wp2, np.float32)
    lnc = _semantic(np.stack([outs[1], outs[2]], axis=1), Wp1, bp1, Wp2)
    dis = _semantic(np.stack([outs[0], outs[3]], axis=1), Wp1, bp1, Wp2)
    return np.stack([lnc, dis]).astype(np.float32)


# revision 18
# speedup vs baseline: 3408.9899x; 1.1262x over previous
"""HAN layer (4-metapath GAT + semantic attention) on 8 Trainium2 NeuronCores.

Sharding: core c handles metapath m = c % 4 and node-half h = c // 4
(235 blocks of 128 nodes; h=0 -> blocks 0..118, h=1 -> blocks 118..235 + dummy).

Everything heavy runs on device in ONE NEFF per core:
 - feature projection feat = hs @ W (bf16, tensor engine) -> fx in DRAM
 - per node-block: one indirect dma_gather pulls feat[src] for all the
   block's (sorted, padded) edges as [128, T, 256] (edge j -> partition
   j%128, slice j//128)
 - one-hot scatter matmuls: acc[n, :] += sum_e onehot[e,n] * (feat[src_e]
   * P_e | P_e), accumulating both messages and softmax denominators in a
   single PSUM tile ([128, 260])
 - epilogue: out = leaky_relu(acc/den + bias) on the vector engine

Host does only O(E) index prep: edge attention logits el/er (tiny matmuls),
P = exp(leaky_relu(el[src]+er[dst])), sort by dst, pack per-block layouts.
Softmax max-shift is skipped: alpha = P/sum(P) is shift-invariant and |e|<~3.
Semantic attention (beta softmax over 2 metapaths + combine) runs on host.
"""
import sys
import numpy as np

sys.path.insert(0, "/opt/trn_rl_repo")

N, E, IN, H, D = 30000, 300000, 128, 4, 64
HD = H * D                      # 256
M = 4                           # metapaths
NCORES = 8
P = 128
NBLK = (N + P - 1) // P         # 235 node blocks total
NPAD = NBLK * P                 # 30080
NB = (NBLK + 1) // 2            # 118 blocks per core
T_MIN = 12                      # min tiles (of 128 edges) per block
NEG_ATTN = 0.2
NEG_ACT = 0.01

_NC_CACHE = {}


def _build_nc(Tb, nb=NB, nblk=NBLK, npad=NPAD):
    """One-core program; same NEFF runs SPMD on all 8 cores.

    Tb: per-block-index tile counts (128 edges per tile), the max over the
    8 cores — pad descriptors cost as much as real ones, so block slots are
    sized to what the cores actually need rather than a uniform worst case.
    """
    import concourse.bacc as bacc
    import concourse.tile as tile
    from concourse import mybir
    from contextlib import ExitStack

    f32 = mybir.dt.float32
    bf16 = mybir.dt.bfloat16
    i16 = mybir.dt.int16
    AOP = mybir.AluOpType

    assert len(Tb) == nb
    NT = sum(Tb)                      # total tiles across blocks
    off = [0]
    for t in Tb:
        off.append(off[-1] + t)

    nc = bacc.Bacc()
    hsT = nc.declare_dram_parameter("hsT", (P, npad), bf16, isOutput=False)
    Wm = nc.declare_dram_parameter("Wm", (P, HD), bf16, isOutput=False)
    idx = nc.declare_dram_parameter("idx", (P, NT * 8), i16, isOutput=False)
    Pb = nc.declare_dram_parameter("Pb", (P, NT * H), bf16, isOutput=False)
    dstl = nc.declare_dram_parameter("dstl", (P, NT), bf16, isOutput=False)
    brep = nc.declare_dram_parameter("brep", (P, HD), f32, isOutput=False)
    iotaf = nc.declare_dram_parameter("iotaf", (P, P), bf16, isOutput=False)
    outp = nc.declare_dram_parameter("outp", (nb * P, HD), f32, isOutput=True)

    with tile.TileContext(nc) as tc, ExitStack() as ctx:
        const = ctx.enter_context(tc.tile_pool(name="const", bufs=1))
        dram = ctx.enter_context(tc.tile_pool(name="dram", bufs=1, space="DRAM"))
        sb = ctx.enter_context(tc.tile_pool(name="sb", bufs=3))
        # blocks span up to 3 gather groups; 4 bufs keeps one group in flight
        gp = ctx.enter_context(tc.tile_pool(name="gp", bufs=4))
        ps = ctx.enter_context(tc.tile_pool(name="ps", bufs=2, space="PSUM"))

        fx = dram.tile([npad, HD], bf16, tag="fx")

        w_sb = const.tile([P, HD], bf16, tag="w")
        nc.sync.dma_start(out=w_sb[:], in_=Wm[:, :])
        iota_sb = const.tile([P, P], bf16, tag="iota")
        nc.sync.dma_start(out=iota_sb[:], in_=iotaf[:, :])
        brep_sb = const.tile([P, HD], f32, tag="brep")
        nc.sync.dma_start(out=brep_sb[:], in_=brep[:, :])
        idx_sb = const.tile([P, NT * 8], i16, tag="idx")
        nc.sync.dma_start(out=idx_sb[:], in_=idx[:, :])
        pb_sb = const.tile([P, NT * H], bf16, tag="pb")
        nc.sync.dma_start(out=pb_sb[:], in_=Pb[:, :])
        dl_sb = const.tile([P, NT], bf16, tag="dl")
        nc.sync.dma_start(out=dl_sb[:], in_=dstl[:, :])
        hsT_sb = const.tile([P, npad], bf16, tag="hsT")
        # split the 60KB/partition load into chunks so it pipelines
        hchunk = -(-npad // 5)
        for k0 in range(0, npad, hchunk):
            k1 = min(k0 + hchunk, npad)
            nc.sync.dma_start(out=hsT_sb[:, k0:k1], in_=hsT[:, k0:k1])

        # ---- phase A: projection feat = hsT^T @ W -> fx (bf16) ----
        SGRP = 4                          # proj tiles per staging buffer
        for g in range(nblk // SGRP + (1 if nblk % SGRP else 0)):
            n_in_g = min(SGRP, nblk - g * SGRP)
            stg = sb.tile([P, SGRP, HD], bf16, tag="stg")
            for j in range(n_in_g):
                i = g * SGRP + j
                pacc = ps.tile([P, HD], f32, tag="pacc")
                nc.tensor.matmul(out=pacc[:], lhsT=hsT_sb[:, i * P:(i + 1) * P],
                                 rhs=w_sb[:], start=True, stop=True)
                if i % 2 == 0:
                    nc.scalar.copy(out=stg[:, j, :], in_=pacc[:])
                else:
                    nc.vector.tensor_copy(out=stg[:, j, :], in_=pacc[:])
            rows = fx[g * SGRP * P:(g * SGRP + n_in_g) * P, :]
            nc.sync.dma_start(
                out=rows.rearrange("(j p) d -> p j d", p=P),
                in_=stg[:, 0:n_in_g, :])

        # ---- phase B: stream of 8-tile gather groups + per-block matmuls ----
        # Gathers are emitted as uniform 1024-descriptor groups over the
        # packed tile stream (descriptor carveout limit), decoupled from
        # block boundaries; onehot/rhs vector ops run per group; the PSUM
        # accumulation + epilogue stay per block.
        GRP = 8
        ngrp = -(-NT // GRP)
        grp = {}

        def emit_group(g):
            k0 = g * GRP
            gl = min(GRP, NT - k0)
            G = gp.tile([P, GRP, HD], bf16, tag="G")
            nc.gpsimd.dma_gather(
                out_ap=G[:, 0:gl, :], in_ap=fx[:, :],
                idxs_ap=idx_sb[:, k0 * 8:(k0 + gl) * 8],
                num_idxs=gl * P, num_idxs_reg=gl * P, elem_size=HD)
            oh = gp.tile([P, GRP, P], bf16, tag="oh")
            nc.vector.tensor_tensor(
                out=oh[:, 0:gl, :],
                in0=iota_sb[:, :].unsqueeze(1).broadcast_to([P, gl, P]),
                in1=dl_sb[:, k0:k0 + gl].unsqueeze(2).broadcast_to([P, gl, P]),
                op=AOP.is_equal)
            rhs = gp.tile([P, GRP, HD + H], bf16, tag="rhs")
            nc.vector.tensor_tensor(
                out=rhs[:, 0:gl, 0:HD].rearrange("p t (h d) -> p t h d", h=H),
                in0=G[:, 0:gl, :].rearrange("p t (h d) -> p t h d", h=H),
                in1=pb_sb[:, k0 * H:(k0 + gl) * H]
                    .rearrange("p (t h) -> p t h", t=gl)
                    .unsqueeze(3).broadcast_to([P, gl, H, D]),
                op=AOP.mult)
            nc.scalar.copy(
                out=rhs[:, 0:gl, HD:HD + H],
                in_=pb_sb[:, k0 * H:(k0 + gl) * H]
                    .rearrange("p (t h) -> p t h", t=gl))
            grp[g] = (oh, rhs)

        emitted = -1
        for i in range(nb):
            T = Tb[i]
            o = off[i]
            while emitted < (o + T - 1) // GRP:
                emitted += 1
                emit_group(emitted)
            acc = ps.tile([P, HD + H], f32, tag="acc")
            for t in range(T):
                k = o + t
                oh, rhs = grp[k // GRP]
                nc.tensor.matmul(out=acc[:], lhsT=oh[:, k % GRP, :],
                                 rhs=rhs[:, k % GRP, :],
                                 start=(t == 0), stop=(t == T - 1))

            den = sb.tile([P, H], f32, tag="den")
            nc.vector.tensor_scalar(out=den[:], in0=acc[:, HD:HD + H],
                                    scalar1=1e-9, scalar2=None, op0=AOP.max)
            rden = sb.tile([P, H], f32, tag="rden")
            nc.vector.reciprocal(out=rden[:], in_=den[:])
            outb = sb.tile([P, HD], f32, tag="outb")
            for h in range(H):
                nc.vector.scalar_tensor_tensor(
                    out=outb[:, h * D:(h + 1) * D],
                    in0=acc[:, h * D:(h + 1) * D],
                    scalar=rden[:, h:h + 1],
                    in1=brep_sb[:, h * D:(h + 1) * D],
                    op0=AOP.mult, op1=AOP.add)
            outb2 = sb.tile([P, HD], f32, tag="outb2")
            nc.vector.scalar_tensor_tensor(
                out=outb2[:], in0=outb[:], scalar=NEG_ACT, in1=outb[:],
                op0=AOP.mult, op1=AOP.max)
            nc.sync.dma_start(out=outp[i * P:(i + 1) * P, :], in_=outb2[:])

    nc.compile()
    return nc


def _get_nc(Tb):
    if Tb not in _NC_CACHE:
        _NC_CACHE[Tb] = _build_nc(Tb)
    return _NC_CACHE[Tb]


def _attn_mat(a):
    """[H, D] head vectors -> [HD, H] block-diagonal matrix."""
    A = np.zeros((HD, H), np.float32)
    for h in range(H):
        A[h * D:(h + 1) * D, h] = a[h]
    return A


def _prep_metapath(hs_m, src_m, dst_m, W_m, al_m, ar_m):
    """Edge exp-weights + dst-sorted edge arrays for one metapath."""
    Wel = (W_m @ _attn_mat(al_m)).astype(np.float32)     # [IN, H]
    Wer = (W_m @ _attn_mat(ar_m)).astype(np.float32)
    el = hs_m @ Wel                                       # [N, H]
    er = hs_m @ Wer
    e = el[src_m] + er[dst_m]                             # [E, H]
    e = np.where(e > 0, e, NEG_ATTN * e)
    Pw = np.exp(e).astype(np.float32)
    order = np.argsort(dst_m, kind="stable")
    ss = src_m[order].astype(np.int64)
    ds = dst_m[order].astype(np.int64)
    Ps = Pw[order]
    blk = ds >> 7
    counts = np.bincount(blk, minlength=NBLK)
    starts = np.concatenate([[0], np.cumsum(counts)[:-1]])
    return ss, ds, Ps, counts, starts


def _pack_core(ss, ds, Ps, counts, starts, blocks, Tb):
    """Device-layout inputs for one core's list of node blocks (variable Tb)."""
    import ml_dtypes
    bf16 = ml_dtypes.bfloat16
    NT = sum(Tb)
    idx16 = np.zeros((16, NT * 8), np.int16)
    Pt = np.zeros((P, NT * H), np.float32)
    dlt = np.zeros((P, NT), np.float32)
    o = 0
    for i, b in enumerate(blocks):
        T = Tb[i]
        SLOTS = T * P
        src_pad = np.zeros(SLOTS, np.int64)
        P_pad = np.zeros((SLOTS, H), np.float32)
        dl_pad = np.zeros(SLOTS, np.float32)
        if b is not None:
            c = counts[b]
            s0 = starts[b]
            src_pad[:c] = ss[s0:s0 + c]
            P_pad[:c] = Ps[s0:s0 + c]
            dl_pad[:c] = ds[s0:s0 + c] - (b << 7)
        # gather idx: index j -> (partition j%16, col j//16)
        idx16[:, o * 8:(o + T) * 8] = src_pad.reshape(T * 8, 16).T
        # edge j -> (partition j%128, tile j//128)
        Pt[:, o * H:(o + T) * H] = (
            P_pad.reshape(T, P, H).transpose(1, 0, 2).reshape(P, T * H))
        dlt[:, o:o + T] = dl_pad.reshape(T, P).T
        o += T
    idx16 = np.tile(idx16, (8, 1)).astype(np.int16)
    return (np.ascontiguousarray(idx16),
            np.ascontiguousarray(Pt.astype(bf16)),
            np.ascontiguousarray(dlt.astype(bf16)))


def _run_device(hs, src, dst, W, attn_l, attn_r, bias, trace=False):
    import ml_dtypes
    from concourse.bass_utils import run_bass_kernel_spmd
    bf16 = ml_dtypes.bfloat16

    preps = [_prep_metapath(np.asarray(hs[m], np.float32), src[m], dst[m],
                            np.asarray(W[m], np.float32),
                            np.asarray(attn_l[m]), np.asarray(attn_r[m]))
             for m in range(M)]
    core_blocks = []
    for c in range(NCORES):
        h = c // M
        blocks = (list(range(0, NB)) if h == 0
                  else list(range(NB, NBLK)) + [None])
        # sort by descending edge count: Tb[i] is a max over the 8 cores,
        # so aligning each core's big blocks at the same index keeps the
        # maxima (and thus pad descriptors) tight
        counts = preps[c % M][3]
        blocks.sort(key=lambda b: -1 if b is None else int(counts[b]),
                    reverse=True)
        core_blocks.append(blocks)
    # per-block-index tile count = max need across the 8 cores
    Tb = []
    for i in range(NB):
        mx = 1
        for c in range(NCORES):
            b = core_blocks[c][i]
            if b is not None:
                mx = max(mx, -(-int(preps[c % M][3][b]) // P))
        Tb.append(mx)
    Tb = tuple(Tb)
    nc = _get_nc(Tb)

    iota = np.ascontiguousarray(
        np.tile(np.arange(P, dtype=np.float32), (P, 1)).astype(bf16))
    in_maps = []
    for c in range(NCORES):
        m, h = c % M, c // M
        ss, ds, Ps, counts, starts = preps[m]
        idx16, Pt, dlt = _pack_core(ss, ds, Ps, counts, starts,
                                    core_blocks[c], Tb)
        hsT = np.zeros((P, NPAD), np.float32)
        hsT[:, :N] = np.asarray(hs[m], np.float32).T
        in_maps.append({
            "hsT": np.ascontiguousarray(hsT.astype(bf16)),
            "Wm": np.ascontiguousarray(np.asarray(W[m]).astype(bf16)),
            "idx": idx16, "Pb": Pt, "dstl": dlt,
            "brep": np.ascontiguousarray(
                np.tile(np.asarray(bias[m], np.float32), (P, 1))),
            "iotaf": iota,
        })
    kw = {}
    if trace:
        kw = dict(trace=True, trace_cores=list(range(NCORES)))
    res = run_bass_kernel_spmd(nc, in_maps, list(range(NCORES)), **kw)
    outs = []
    for m in range(M):
        out_m = np.zeros((NPAD, HD), np.float32)
        for c in (m, m + 4):
            rows = res.results[c]["outp"].reshape(NB, P, HD)
            bids = [b for b in core_blocks[c] if b is not None]
            iidx = [i for i, b in enumerate(core_blocks[c]) if b is not None]
            out_m.reshape(NBLK, P, HD)[bids] = rows[iidx]
        outs.append(out_m[:N])
    return outs, res


def _semantic(z, Wp1, bp1, Wp2):
    w = (np.tanh(z @ Wp1 + bp1) @ Wp2).mean(0)            # [2, 1]
    w = w - w.max()
    beta = np.exp(w) / np.exp(w).sum()
    return (beta[None] * z).sum(1)


def kernel(hs, src, dst, W, attn_l, attn_r, bias, Wp1, bp1, Wp2):
    hs = np.asarray(hs, np.float32)
    src = np.asarray(src)
    dst = np.asarray(dst)
    W = np.asarray(W, np.float32)

    outs, _ = _run_device(hs, src, dst, W, attn_l, attn_r, bias)

    Wp1 = np.asarray(Wp1, np.float32)
    bp1 = np.asarray(bp1, np.float32)
    Wp2 = np.asarray(Wp2, np.float32)
    lnc = _semantic(np.stack([outs[1], outs[2]], axis=1), Wp1, bp1, Wp2)
    dis = _semantic(np.stack([outs[0], outs[3]], axis=1), Wp1, bp1, Wp2)
    return np.stack([lnc, dis]).astype(np.float32)
